# revision 4
# baseline (speedup 1.0000x reference)
"""Dense MoE (BasicMoE) Trainium2 Bass kernel.

Problem (hardcoded): x [4, 2048, 1024] f32, gate_w [1024, 8], gate_b [8],
expert_w [8, 1024, 1024], expert_b [8, 1024].

    tok = x.reshape(T, H)
    w   = softmax(tok @ gate_w + gate_b)           # [T, E]
    eo  = einsum('th,ehd->ted', tok, expert_w) + expert_b
    out = einsum('te,ted->td', w, eo)              # [T, H]

Sharding: tokens split across 8 cores (data parallel), weights replicated.

Per-core algorithm (T_l = 1024 tokens):
  1. Transpose x shard on the PE -> xT (h on partitions), needed because the
     TensorEngine contracts along the partition dim.
  2. Gate: logits via matmul (xT slices stationary, gate_w moving), then
     ew = exp(logits + gate_b) WITHOUT normalization; S = sum_e ew;
     the 1/S is applied once at the very end (softmax is linear in the
     combine, so the division can be deferred past the weighted sum).
  3. acc[t,d] is seeded with the bias term sum_e ew[t,e]*b_e[d] (a K=8
     matmul against ewT), then for each expert: y_e = xT.T @ W_e accumulated
     over k in PSUM (float32r matmuls: full PE rate), evicted by the scalar
     engine as y_e * ew[:,e] (per-partition scale) and added into acc by DVE.
  4. out = acc * (1/S) (per-partition scale), DMA to DRAM.
"""

import os
from contextlib import ExitStack

import numpy as np

import concourse.bass as bass
import concourse.tile as tile
from concourse import bacc, mybir
from concourse.bass_utils import run_bass_kernel_spmd
from concourse.masks import make_identity

B, S, H, E = 4, 2048, 1024, 8
T = B * S
N_CORES = 8
TL = T // N_CORES          # tokens per core = 1024
P = 128                    # SBUF partitions
KT = H // P                # 8 contraction tiles
MT = TL // P               # 8 token tiles per core
DH = 512                   # matmul moving free-dim (fp32 PSUM bank)
ND = H // DH               # 2 d-halves

F32 = mybir.dt.float32
F32R = mybir.dt.float32r

_CACHE = {}
LAST_RESULT = None


def _r(ap):
    """Bitcast an f32 AP to float32r for full-rate PE matmuls."""
    return ap.bitcast(F32R)


def _build_moe_nc():
    nc = bacc.Bacc(
        "TRN2",
        target_bir_lowering=False,
        debug=False,
        enable_asserts=False,
        num_devices=N_CORES,
    )

    x_sh = nc.dram_tensor("x_sh", [TL, H], F32, kind="ExternalInput").ap()
    gate_w = nc.dram_tensor("gate_w", [H, E], F32, kind="ExternalInput").ap()
    gate_b = nc.dram_tensor("gate_b", [E], F32, kind="ExternalInput").ap()
    expert_w = nc.dram_tensor("expert_w", [E, H, H], F32, kind="ExternalInput").ap()
    expert_b = nc.dram_tensor("expert_b", [E, H], F32, kind="ExternalInput").ap()
    out_sh = nc.dram_tensor("out_sh", [TL, H], F32, kind="ExternalOutput").ap()

    with tile.TileContext(nc) as tc, ExitStack() as ctx:
        const = ctx.enter_context(tc.tile_pool(name="const", bufs=1))
        xload = ctx.enter_context(tc.tile_pool(name="xload", bufs=3))
        wpool = ctx.enter_context(tc.tile_pool(name="wpool", bufs=2))
        accp = ctx.enter_context(tc.tile_pool(name="accp", bufs=1))
        tmp = ctx.enter_context(tc.tile_pool(name="tmp", bufs=6))
        outp = ctx.enter_context(tc.tile_pool(name="outp", bufs=3))
        psum = ctx.enter_context(tc.tile_pool(name="psum", bufs=4, space="PSUM"))
        psum_s = ctx.enter_context(tc.tile_pool(name="psum_s", bufs=1, space="PSUM"))

        # ---- constants -------------------------------------------------
        ident = const.tile([P, P], F32)
        make_identity(nc, ident)

        # gate_w as [p, k, e]
        gw = const.tile([P, KT, E], F32R)
        for k in range(KT):
            nc.sync.dma_start(gw[:, k, :], _r(gate_w[k * P : (k + 1) * P, :]))
        # gate_b broadcast across partitions -> [P, E]
        gb = const.tile([P, E], F32)
        gate_b_bcast = bass.AP(
            tensor=gate_b.tensor, offset=gate_b.offset, ap=[[0, P]] + list(gate_b.ap)
        )
        nc.sync.dma_start(gb, gate_b_bcast)
        # expert_b natural layout [E, H] (e on partitions)
        eb = const.tile([E, H], F32R)
        nc.sync.dma_start(eb, _r(expert_b))

        # ---- resident per-core state ----------------------------------
        xT = [const.tile([P, TL], F32R, name=f"xT{k}") for k in range(KT)]
        ew = const.tile([P, MT, E], F32)      # unnormalized softmax weights
        ewT = const.tile([E, TL], F32R)        # transposed gate weights
        invS = const.tile([P, MT], F32)       # 1/sum_e ew per token
        acc = [accp.tile([P, H], F32, name=f"acc{m}") for m in range(MT)]

        # ---- phase 1: load x, transpose, gate -------------------------
        for m in range(MT):
            xrow = xload.tile([P, H], F32)
            nc.sync.dma_start(xrow, x_sh[m * P : (m + 1) * P, :])
            msl = slice(m * P, (m + 1) * P)
            for k in range(KT):
                pt = psum_s.tile([P, P], F32, tag="pt", bufs=2)
                nc.tensor.transpose(pt, xrow[:, k * P : (k + 1) * P], ident)
                nc.vector.tensor_copy(xT[k][:, msl], _r(pt))

            # gate logits for this token tile: [P, E]
            pg = psum_s.tile([P, E], F32, tag="pg", bufs=1)
            for k in range(KT):
                nc.tensor.matmul(
                    pg,
                    lhsT=xT[k][:, msl],
                    rhs=gw[:, k, :],
                    start=(k == 0),
                    stop=(k == KT - 1),
                )
            lg = tmp.tile([P, E], F32, tag="lg")
            nc.vector.tensor_add(lg, pg, gb)
            nc.scalar.activation(ew[:, m, :], lg, mybir.ActivationFunctionType.Exp)
            ssum = tmp.tile([P, 1], F32, tag="ssum")
            nc.vector.reduce_sum(ssum, ew[:, m, :], axis=mybir.AxisListType.X)
            nc.vector.reciprocal(invS[:, m : m + 1], ssum)
            # ewT[:, m*P:(m+1)*P] = ew[:, m, :].T
            pt2 = psum_s.tile([E, P], F32, tag="pt2", bufs=1)
            nc.tensor.transpose(pt2, ew[:, m, :], ident)
            nc.vector.tensor_copy(ewT[:, msl], _r(pt2))

        # ---- phase 2: seed acc with bias term -------------------------
        # acc[t, d] = sum_e ew[t, e] * expert_b[e, d]   (K=8 matmul)
        for m in range(MT):
            msl = slice(m * P, (m + 1) * P)
            for n in range(ND):
                nsl = slice(n * DH, (n + 1) * DH)
                pb = psum.tile([P, DH], F32, tag="ps")
                nc.tensor.matmul(
                    pb, lhsT=ewT[:, msl], rhs=eb[:, nsl],
                    start=True, stop=True,
                )
                nc.vector.tensor_copy(acc[m][:, nsl], pb)

        # ---- phase 3: experts -----------------------------------------
        for e in range(E):
            wsb = wpool.tile([P, KT, H], F32R, tag="w")
            for k in range(KT):
                nc.sync.dma_start(
                    wsb[:, k, :], _r(expert_w[e, k * P : (k + 1) * P, :])
                )
            for m in range(MT):
                msl = slice(m * P, (m + 1) * P)
                for n in range(ND):
                    nsl = slice(n * DH, (n + 1) * DH)
                    ps = psum.tile([P, DH], F32, tag="ps")
                    for k in range(KT):
                        nc.tensor.matmul(
                            ps,
                            lhsT=xT[k][:, msl],
                            rhs=wsb[:, k, nsl],
                            start=(k == 0),
                            stop=(k == KT - 1),
                        )
                    # evict scaled by gate weight (per-partition scalar)
                    t = tmp.tile([P, DH], F32, tag="evict")
                    nc.scalar.mul(t, ps, ew[:, m, e : e + 1])
                    nc.vector.tensor_add(acc[m][:, nsl], acc[m][:, nsl], t)

        # ---- phase 4: normalize + store -------------------------------
        for m in range(MT):
            ob = outp.tile([P, H], F32, tag="ob")
            nc.scalar.mul(ob, acc[m], invS[:, m : m + 1])
            nc.sync.dma_start(out_sh[m * P : (m + 1) * P, :], ob)

    nc.compile()
    return nc


def kernel(**inputs) -> np.ndarray:
    global LAST_RESULT
    x = np.ascontiguousarray(np.asarray(inputs["x"], dtype=np.float32)).reshape(T, H)
    gw = np.ascontiguousarray(np.asarray(inputs["gate_w"], dtype=np.float32))
    gb = np.ascontiguousarray(np.asarray(inputs["gate_b"], dtype=np.float32))
    ew = np.ascontiguousarray(np.asarray(inputs["expert_w"], dtype=np.float32))
    eb = np.ascontiguousarray(np.asarray(inputs["expert_b"], dtype=np.float32))

    if "nc" not in _CACHE:
        _CACHE["nc"] = _build_moe_nc()
    nc = _CACHE["nc"]

    in_maps = [
        {
            "x_sh": x[c * TL : (c + 1) * TL],
            "gate_w": gw,
            "gate_b": gb,
            "expert_w": ew,
            "expert_b": eb,
        }
        for c in range(N_CORES)
    ]
    res = run_bass_kernel_spmd(
        nc,
        in_maps,
        core_ids=list(range(N_CORES)),
        trace=bool(int(os.environ.get("MOE_TRACE", "0"))),
    )
    LAST_RESULT = res
    out = np.concatenate([res.results[c]["out_sh"] for c in range(N_CORES)], axis=0)
    return out.reshape(B, S, H)


# revision 7
# speedup vs baseline: 1.0248x; 1.0248x over previous
"""Dense MoE (BasicMoE) Trainium2 Bass kernel.

Problem (hardcoded): x [4, 2048, 1024] f32, gate_w [1024, 8], gate_b [8],
expert_w [8, 1024, 1024], expert_b [8, 1024].

    tok = x.reshape(T, H)
    w   = softmax(tok @ gate_w + gate_b)           # [T, E]
    eo  = einsum('th,ehd->ted', tok, expert_w) + expert_b
    out = einsum('te,ted->td', w, eo)              # [T, H]

Sharding: tokens split across 8 cores (data parallel), weights replicated.

Per-core algorithm (T_l = 1024 tokens). The TensorEngine contracts along
the partition dim, so the activations are needed h-major (xT); that
transpose is pure data layout, done host-side when sharding.

  1. Gate, in transposed [e, t] layout: logitsT = gate_w.T @ x.T via
     matmuls with gate_w slices stationary (8-wide LDWEIGHTS);
     ewT = exp(logitsT + gate_b) with gate_b as a natural per-partition
     ACT bias. Small PE transposes give ew in [t, e] layout, where
     S = sum_e ew and ews = ew * (1/S) are per-partition ops. softmax's
     division is linear in the combine, so it is folded into the gate
     weights and nothing needs normalizing at the end. ews is transposed
     back (ewsT) for the bias term.
  2. acc[t,d] seeded with the bias term sum_e ews[t,e]*b_e[d] (K=8 matmul
     of ewsT against expert_b).
  3. For each expert: y_e = xT.T @ W_e accumulated over k in PSUM
     (float32r matmuls: full PE rate), evicted by the scalar engine as
     y_e * ews[:,e] (per-partition scale) and added into acc by DVE.
  4. acc IS the output: DMA out per 512-wide half as soon as the last
     expert's contribution lands.
"""

import os
from contextlib import ExitStack

import numpy as np

import concourse.bass as bass
import concourse.tile as tile
from concourse import bacc, mybir
from concourse.bass_utils import run_bass_kernel_spmd
from concourse.masks import make_identity

B, S, H, E = 4, 2048, 1024, 8
T = B * S
N_CORES = 8
TL = T // N_CORES          # tokens per core = 1024
P = 128                    # SBUF partitions
KT = H // P                # 8 contraction tiles
MT = TL // P               # 8 token tiles per core
DH = 512                   # matmul moving free-dim (fp32 PSUM bank)
ND = H // DH               # 2 d-halves
XC = 4                     # x DMA column chunks (queue parallelism)
WC = 2                     # expert_w DMA column chunks
OC = 2                     # output DMA column chunks per (m, half)

F32 = mybir.dt.float32
F32R = mybir.dt.float32r

_CACHE = {}
LAST_RESULT = None


def _r(ap):
    """Bitcast an f32 AP to float32r (same bits; PE rounds internally)."""
    return ap.bitcast(F32R)


def _build_moe_nc():
    nc = bacc.Bacc(
        "TRN2",
        target_bir_lowering=False,
        debug=False,
        enable_asserts=False,
        num_devices=N_CORES,
    )

    x_shT = nc.dram_tensor("x_shT", [H, TL], F32, kind="ExternalInput").ap()
    gate_w = nc.dram_tensor("gate_w", [H, E], F32, kind="ExternalInput").ap()
    gate_b = nc.dram_tensor("gate_b", [E], F32, kind="ExternalInput").ap()
    expert_w = nc.dram_tensor("expert_w", [E, H, H], F32, kind="ExternalInput").ap()
    expert_b = nc.dram_tensor("expert_b", [E, H], F32, kind="ExternalInput").ap()
    out_sh = nc.dram_tensor("out_sh", [TL, H], F32, kind="ExternalOutput").ap()

    with tile.TileContext(nc) as tc, ExitStack() as ctx:
        const = ctx.enter_context(tc.tile_pool(name="const", bufs=1))
        wpool = ctx.enter_context(tc.tile_pool(name="wpool", bufs=2))
        accp = ctx.enter_context(tc.tile_pool(name="accp", bufs=1))
        tmp = ctx.enter_context(tc.tile_pool(name="tmp", bufs=6))
        psum = ctx.enter_context(tc.tile_pool(name="psum", bufs=6, space="PSUM"))
        psum_s = ctx.enter_context(tc.tile_pool(name="psum_s", bufs=1, space="PSUM"))

        ident = const.tile([P, P], F32)
        make_identity(nc, ident)

        # ---- loads ------------------------------------------------------
        # xT: h on partitions, t on free — straight (contiguous) DMA from
        # the host-transposed shard, column-chunked for queue parallelism.
        xT = [const.tile([P, TL], F32R, name=f"xT{k}") for k in range(KT)]
        xcw = TL // XC
        for k in range(KT):
            for c in range(XC):
                csl = slice(c * xcw, (c + 1) * xcw)
                nc.sync.dma_start(
                    xT[k][:, csl], _r(x_shT[k * P : (k + 1) * P, csl])
                )

        gw = const.tile([P, KT, E], F32R)
        for k in range(KT):
            nc.sync.dma_start(gw[:, k, :], _r(gate_w[k * P : (k + 1) * P, :]))
        gb8 = const.tile([E, 1], F32)
        nc.sync.dma_start(gb8, gate_b[:, None])
        eb = const.tile([E, H], F32R)
        nc.sync.dma_start(eb, _r(expert_b))

        # ---- gate -------------------------------------------------------
        ewT_raw = const.tile([E, TL], F32)    # exp(logits).T (unnormalized)
        ews = const.tile([P, MT, E], F32)     # per-token gate weight / S
        ewsT = const.tile([E, TL], F32R)      # ews transposed, for bias mm

        for h2 in range(2):
            hsl = slice(h2 * DH, (h2 + 1) * DH)
            pgT = psum_s.tile([E, DH], F32, tag="sm", bufs=2)
            for k in range(KT):
                nc.tensor.matmul(
                    pgT,
                    lhsT=gw[:, k, :],
                    rhs=xT[k][:, hsl],
                    start=(k == 0),
                    stop=(k == KT - 1),
                )
            # ewT = exp(logitsT + gate_b); gate_b is per-partition here
            nc.scalar.activation(
                ewT_raw[:, hsl], pgT, mybir.ActivationFunctionType.Exp, bias=gb8
            )

        for m in range(MT):
            msl = slice(m * P, (m + 1) * P)
            # ew[t, e] for this token tile via PE transpose
            ptw = psum_s.tile([P, E], F32, tag="sm", bufs=2)
            nc.tensor.transpose(ptw, ewT_raw[:, msl], ident[:E, :E])
            ssum = tmp.tile([P, 1], F32, tag="ssum")
            nc.vector.reduce_sum(ssum, ptw, axis=mybir.AxisListType.X)
            inv = tmp.tile([P, 1], F32, tag="inv")
            nc.vector.reciprocal(inv, ssum)
            nc.vector.tensor_scalar_mul(ews[:, m, :], ptw, inv)
            # back-transpose the normalized weights for the bias matmul
            ptb = psum_s.tile([E, P], F32, tag="sm", bufs=2)
            nc.tensor.transpose(ptb, ews[:, m, :], ident)
            nc.vector.tensor_copy(ewsT[:, msl], _r(ptb))

        # ---- bias seed: acc = ews @ expert_b ---------------------------
        acc = [accp.tile([P, H], F32, name=f"acc{m}") for m in range(MT)]
        for m in range(MT):
            msl = slice(m * P, (m + 1) * P)
            for n in range(ND):
                nsl = slice(n * DH, (n + 1) * DH)
                pb = psum.tile([P, DH], F32, tag="ps")
                nc.tensor.matmul(
                    pb, lhsT=ewsT[:, msl], rhs=eb[:, nsl], start=True, stop=True
                )
                nc.vector.tensor_copy(acc[m][:, nsl], pb)

        # ---- experts ----------------------------------------------------
        ocw = DH // OC
        for e in range(E):
            wsb = wpool.tile([P, KT, H], F32R, tag="w")
            wcw = H // WC
            for k in range(KT):
                for c in range(WC):
                    csl = slice(c * wcw, (c + 1) * wcw)
                    nc.sync.dma_start(
                        wsb[:, k, csl],
                        _r(expert_w[e, k * P : (k + 1) * P, csl]),
                    )
            last = e == E - 1
            for m in range(MT):
                msl = slice(m * P, (m + 1) * P)
                for n in range(ND):
                    nsl = slice(n * DH, (n + 1) * DH)
                    ps = psum.tile([P, DH], F32, tag="ps")
                    for k in range(KT):
                        nc.tensor.matmul(
                            ps,
                            lhsT=xT[k][:, msl],
                            rhs=wsb[:, k, nsl],
                            start=(k == 0),
                            stop=(k == KT - 1),
                        )
                    # evict scaled by normalized gate weight
                    t = tmp.tile([P, DH], F32, tag="evict")
                    nc.scalar.mul(t, ps, ews[:, m, e : e + 1])
                    nc.vector.tensor_add(acc[m][:, nsl], acc[m][:, nsl], t)
                    if last:
                        for c in range(OC):
                            osl = slice(
                                n * DH + c * ocw, n * DH + (c + 1) * ocw
                            )
                            nc.sync.dma_start(
                                out_sh[m * P : (m + 1) * P, osl],
                                acc[m][:, osl],
                            )

    nc.compile()
    return nc


def kernel(**inputs) -> np.ndarray:
    global LAST_RESULT
    x = np.asarray(inputs["x"], dtype=np.float32).reshape(T, H)
    gw = np.ascontiguousarray(np.asarray(inputs["gate_w"], dtype=np.float32))
    gb = np.ascontiguousarray(np.asarray(inputs["gate_b"], dtype=np.float32))
    ew = np.ascontiguousarray(np.asarray(inputs["expert_w"], dtype=np.float32))
    eb = np.ascontiguousarray(np.asarray(inputs["expert_b"], dtype=np.float32))

    if "nc" not in _CACHE:
        _CACHE["nc"] = _build_moe_nc()
    nc = _CACHE["nc"]

    in_maps = [
        {
            "x_shT": np.ascontiguousarray(x[c * TL : (c + 1) * TL].T),
            "gate_w": gw,
            "gate_b": gb,
            "expert_w": ew,
            "expert_b": eb,
        }
        for c in range(N_CORES)
    ]
    res = run_bass_kernel_spmd(
        nc,
        in_maps,
        core_ids=list(range(N_CORES)),
        trace=bool(int(os.environ.get("MOE_TRACE", "0"))),
    )
    LAST_RESULT = res
    out = np.concatenate([res.results[c]["out_sh"] for c in range(N_CORES)], axis=0)
    return out.reshape(B, S, H)


# revision 8
# speedup vs baseline: 1.0650x; 1.0392x over previous
"""Dense MoE (BasicMoE) Trainium2 Bass kernel.

Problem (hardcoded): x [4, 2048, 1024] f32, gate_w [1024, 8], gate_b [8],
expert_w [8, 1024, 1024], expert_b [8, 1024].

    tok = x.reshape(T, H)
    w   = softmax(tok @ gate_w + gate_b)           # [T, E]
    eo  = einsum('th,ehd->ted', tok, expert_w) + expert_b
    out = einsum('te,ted->td', w, eo)              # [T, H]

Sharding: tokens split across 8 cores (data parallel), weights replicated.

Per-core algorithm (T_l = 1024 tokens). The TensorEngine contracts along
the partition dim, so the activations are needed h-major (xT); that
transpose is pure data layout, done host-side when sharding.

  1. Gate, in transposed [e, t] layout: logitsT = gate_w.T @ x.T via
     matmuls with gate_w slices stationary (8-wide LDWEIGHTS);
     ewT = exp(logitsT + gate_b) with gate_b as a natural per-partition
     ACT bias. Small PE transposes give ew in [t, e] layout, where
     S = sum_e ew and ews = ew * (1/S) are per-partition ops. softmax's
     division is linear in the combine, so it is folded into the gate
     weights and nothing needs normalizing at the end. ews is transposed
     back (ewsT) for the bias term.
  2. acc[t,d] seeded with the bias term sum_e ews[t,e]*b_e[d] (K=8 matmul
     of ewsT against expert_b).
  3. For each expert: y_e = xT.T @ W_e accumulated over k in PSUM
     (float32r matmuls: full PE rate), evicted by the scalar engine as
     y_e * ews[:,e] (per-partition scale) and added into acc by DVE.
  4. acc IS the output: DMA out per 512-wide half as soon as the last
     expert's contribution lands.
"""

import os
from contextlib import ExitStack

import numpy as np

import concourse.bass as bass
import concourse.tile as tile
from concourse import bacc, mybir
from concourse.bass_utils import run_bass_kernel_spmd
from concourse.masks import make_identity

B, S, H, E = 4, 2048, 1024, 8
T = B * S
N_CORES = 8
TL = T // N_CORES          # tokens per core = 1024
P = 128                    # SBUF partitions
KT = H // P                # 8 contraction tiles
MT = TL // P               # 8 token tiles per core
DH = 512                   # matmul moving free-dim (fp32 PSUM bank)
ND = H // DH               # 2 d-halves
XC = 2                     # x DMA column chunks (queue parallelism)
WC = 2                     # expert_w DMA column chunks
OC = 2                     # output DMA column chunks per (m, half)

F32 = mybir.dt.float32
F32R = mybir.dt.float32r

_CACHE = {}
LAST_RESULT = None


def _r(ap):
    """Bitcast an f32 AP to float32r (same bits; PE rounds internally)."""
    return ap.bitcast(F32R)


def _build_moe_nc():
    nc = bacc.Bacc(
        "TRN2",
        target_bir_lowering=False,
        debug=False,
        enable_asserts=False,
        num_devices=N_CORES,
    )

    x_shT = nc.dram_tensor("x_shT", [H, TL], F32, kind="ExternalInput").ap()
    gate_w = nc.dram_tensor("gate_w", [H, E], F32, kind="ExternalInput").ap()
    gate_b = nc.dram_tensor("gate_b", [E], F32, kind="ExternalInput").ap()
    expert_w = nc.dram_tensor("expert_w", [E, H, H], F32, kind="ExternalInput").ap()
    expert_b = nc.dram_tensor("expert_b", [E, H], F32, kind="ExternalInput").ap()
    out_sh = nc.dram_tensor("out_sh", [TL, H], F32, kind="ExternalOutput").ap()

    with tile.TileContext(nc) as tc, ExitStack() as ctx:
        const = ctx.enter_context(tc.tile_pool(name="const", bufs=1))
        wpool = ctx.enter_context(tc.tile_pool(name="wpool", bufs=2))
        accp = ctx.enter_context(tc.tile_pool(name="accp", bufs=1))
        tmp = ctx.enter_context(tc.tile_pool(name="tmp", bufs=6))
        psum = ctx.enter_context(tc.tile_pool(name="psum", bufs=6, space="PSUM"))
        psum_s = ctx.enter_context(tc.tile_pool(name="psum_s", bufs=1, space="PSUM"))

        ident = const.tile([P, P], F32)
        make_identity(nc, ident)

        # ---- loads ------------------------------------------------------
        gw = const.tile([P, KT, E], F32R)
        for k in range(KT):
            nc.sync.dma_start(gw[:, k, :], _r(gate_w[k * P : (k + 1) * P, :]))
        gb8 = const.tile([E, 1], F32)
        nc.sync.dma_start(gb8, gate_b[:, None])
        eb = const.tile([E, H], F32R)
        nc.sync.dma_start(eb, _r(expert_b))

        # xT: h on partitions, t on free — straight (contiguous) DMA from the
        # host-transposed shard. GpSimd SWDGE queues, half-column chunks in
        # half-major order so the first gate matmul's operands land first.
        xT = [const.tile([P, TL], F32R, name=f"xT{k}") for k in range(KT)]
        xcw = TL // XC
        for c in range(XC):
            for k in range(KT):
                csl = slice(c * xcw, (c + 1) * xcw)
                nc.gpsimd.dma_start(
                    xT[k][:, csl], _r(x_shT[k * P : (k + 1) * P, csl])
                )

        # ---- gate -------------------------------------------------------
        ewT_raw = const.tile([E, TL], F32)    # exp(logits).T (unnormalized)
        ews = const.tile([P, MT, E], F32)     # per-token gate weight / S
        ewsT = const.tile([E, TL], F32R)      # ews transposed, for bias mm

        for h2 in range(2):
            hsl = slice(h2 * DH, (h2 + 1) * DH)
            pgT = psum_s.tile([E, DH], F32, tag="sm", bufs=2)
            for k in range(KT):
                nc.tensor.matmul(
                    pgT,
                    lhsT=gw[:, k, :],
                    rhs=xT[k][:, hsl],
                    start=(k == 0),
                    stop=(k == KT - 1),
                )
            # ewT = exp(logitsT + gate_b); gate_b is per-partition here
            nc.scalar.activation(
                ewT_raw[:, hsl], pgT, mybir.ActivationFunctionType.Exp, bias=gb8
            )

        for m in range(MT):
            msl = slice(m * P, (m + 1) * P)
            # ew[t, e] for this token tile via PE transpose
            ptw = psum_s.tile([P, E], F32, tag="sm", bufs=2)
            nc.tensor.transpose(ptw, ewT_raw[:, msl], ident[:E, :E])
            ssum = tmp.tile([P, 1], F32, tag="ssum")
            nc.vector.reduce_sum(ssum, ptw, axis=mybir.AxisListType.X)
            inv = tmp.tile([P, 1], F32, tag="inv")
            nc.vector.reciprocal(inv, ssum)
            nc.vector.tensor_scalar_mul(ews[:, m, :], ptw, inv)
            # back-transpose the normalized weights for the bias matmul
            ptb = psum_s.tile([E, P], F32, tag="sm", bufs=2)
            nc.tensor.transpose(ptb, ews[:, m, :], ident)
            nc.vector.tensor_copy(ewsT[:, msl], _r(ptb))

        # ---- bias seed: acc = ews @ expert_b ---------------------------
        acc = [accp.tile([P, H], F32, name=f"acc{m}") for m in range(MT)]
        for m in range(MT):
            msl = slice(m * P, (m + 1) * P)
            for n in range(ND):
                nsl = slice(n * DH, (n + 1) * DH)
                pb = psum.tile([P, DH], F32, tag="ps")
                nc.tensor.matmul(
                    pb, lhsT=ewsT[:, msl], rhs=eb[:, nsl], start=True, stop=True
                )
                nc.vector.tensor_copy(acc[m][:, nsl], pb)

        # ---- experts ----------------------------------------------------
        ocw = DH // OC
        for e in range(E):
            wsb = wpool.tile([P, KT, H], F32R, tag="w")
            wcw = H // WC
            for c in range(WC):
                for k in range(KT):
                    csl = slice(c * wcw, (c + 1) * wcw)
                    nc.sync.dma_start(
                        wsb[:, k, csl],
                        _r(expert_w[e, k * P : (k + 1) * P, csl]),
                    )
            last = e == E - 1
            for m in range(MT):
                msl = slice(m * P, (m + 1) * P)
                for n in range(ND):
                    nsl = slice(n * DH, (n + 1) * DH)
                    ps = psum.tile([P, DH], F32, tag="ps")
                    for k in range(KT):
                        nc.tensor.matmul(
                            ps,
                            lhsT=xT[k][:, msl],
                            rhs=wsb[:, k, nsl],
                            start=(k == 0),
                            stop=(k == KT - 1),
                        )
                    # evict scaled by normalized gate weight
                    t = tmp.tile([P, DH], F32, tag="evict")
                    nc.scalar.mul(t, ps, ews[:, m, e : e + 1])
                    nc.vector.tensor_add(acc[m][:, nsl], acc[m][:, nsl], t)
                    if last:
                        for c in range(OC):
                            osl = slice(
                                n * DH + c * ocw, n * DH + (c + 1) * ocw
                            )
                            nc.gpsimd.dma_start(
                                out_sh[m * P : (m + 1) * P, osl],
                                acc[m][:, osl],
                            )

    nc.compile()
    return nc


def kernel(**inputs) -> np.ndarray:
    global LAST_RESULT
    x = np.asarray(inputs["x"], dtype=np.float32).reshape(T, H)
    gw = np.ascontiguousarray(np.asarray(inputs["gate_w"], dtype=np.float32))
    gb = np.ascontiguousarray(np.asarray(inputs["gate_b"], dtype=np.float32))
    ew = np.ascontiguousarray(np.asarray(inputs["expert_w"], dtype=np.float32))
    eb = np.ascontiguousarray(np.asarray(inputs["expert_b"], dtype=np.float32))

    if "nc" not in _CACHE:
        _CACHE["nc"] = _build_moe_nc()
    nc = _CACHE["nc"]

    in_maps = [
        {
            "x_shT": np.ascontiguousarray(x[c * TL : (c + 1) * TL].T),
            "gate_w": gw,
            "gate_b": gb,
            "expert_w": ew,
            "expert_b": eb,
        }
        for c in range(N_CORES)
    ]
    res = run_bass_kernel_spmd(
        nc,
        in_maps,
        core_ids=list(range(N_CORES)),
        trace=bool(int(os.environ.get("MOE_TRACE", "0"))),
    )
    LAST_RESULT = res
    out = np.concatenate([res.results[c]["out_sh"] for c in range(N_CORES)], axis=0)
    return out.reshape(B, S, H)


# revision 9
# speedup vs baseline: 1.0750x; 1.0094x over previous
"""Dense MoE (BasicMoE) Trainium2 Bass kernel.

Problem (hardcoded): x [4, 2048, 1024] f32, gate_w [1024, 8], gate_b [8],
expert_w [8, 1024, 1024], expert_b [8, 1024].

    tok = x.reshape(T, H)
    w   = softmax(tok @ gate_w + gate_b)           # [T, E]
    eo  = einsum('th,ehd->ted', tok, expert_w) + expert_b
    out = einsum('te,ted->td', w, eo)              # [T, H]

Sharding: tokens split across 8 cores (data parallel), weights replicated.

Per-core algorithm (T_l = 1024 tokens). The TensorEngine contracts along
the partition dim, so the activations are needed h-major (xT); that
transpose is pure data layout, done host-side when sharding.

  1. Gate, in transposed [e, t] layout: logitsT = gate_w.T @ x.T via
     matmuls with gate_w slices stationary (8-wide LDWEIGHTS);
     ewT = exp(logitsT + gate_b) with gate_b as a natural per-partition
     ACT bias. Small PE transposes give ew in [t, e] layout, where
     S = sum_e ew and ews = ew * (1/S) are per-partition ops. softmax's
     division is linear in the combine, so it is folded into the gate
     weights and nothing needs normalizing at the end. ews is transposed
     back (ewsT) for the bias term.
  2. acc[t,d] seeded with the bias term sum_e ews[t,e]*b_e[d] (K=8 matmul
     of ewsT against expert_b).
  3. For each expert: y_e = xT.T @ W_e accumulated over k in PSUM
     (float32r matmuls: full PE rate), evicted by the scalar engine as
     y_e * ews[:,e] (per-partition scale) and added into acc by DVE.
  4. acc IS the output: DMA out per 512-wide half as soon as the last
     expert's contribution lands.
"""

import os
from contextlib import ExitStack

import numpy as np

import concourse.bass as bass
import concourse.tile as tile
from concourse import bacc, mybir
from concourse.bass_utils import run_bass_kernel_spmd
from concourse.masks import make_identity

B, S, H, E = 4, 2048, 1024, 8
T = B * S
N_CORES = 8
TL = T // N_CORES          # tokens per core = 1024
P = 128                    # SBUF partitions
KT = H // P                # 8 contraction tiles
MT = TL // P               # 8 token tiles per core
DH = 512                   # matmul moving free-dim (fp32 PSUM bank)
ND = H // DH               # 2 d-halves
XC = 2                     # x DMA column chunks (queue parallelism)
WC = 2                     # expert_w DMA column chunks
OC = 2                     # output DMA column chunks per (m, half)

F32 = mybir.dt.float32
F32R = mybir.dt.float32r

_CACHE = {}
LAST_RESULT = None


def _r(ap):
    """Bitcast an f32 AP to float32r (same bits; PE rounds internally)."""
    return ap.bitcast(F32R)


def _build_moe_nc():
    nc = bacc.Bacc(
        "TRN2",
        target_bir_lowering=False,
        debug=False,
        enable_asserts=False,
        num_devices=N_CORES,
    )

    x_shT = nc.dram_tensor("x_shT", [H, TL], F32, kind="ExternalInput").ap()
    gate_w = nc.dram_tensor("gate_w", [H, E], F32, kind="ExternalInput").ap()
    gate_b = nc.dram_tensor("gate_b", [E], F32, kind="ExternalInput").ap()
    expert_w = nc.dram_tensor("expert_w", [E, H, H], F32, kind="ExternalInput").ap()
    expert_b = nc.dram_tensor("expert_b", [E, H], F32, kind="ExternalInput").ap()
    out_sh = nc.dram_tensor("out_sh", [TL, H], F32, kind="ExternalOutput").ap()

    with tile.TileContext(nc) as tc, ExitStack() as ctx:
        const = ctx.enter_context(tc.tile_pool(name="const", bufs=1))
        wpool = ctx.enter_context(tc.tile_pool(name="wpool", bufs=2))
        accp = ctx.enter_context(tc.tile_pool(name="accp", bufs=1))
        tmp = ctx.enter_context(tc.tile_pool(name="tmp", bufs=6))
        psum = ctx.enter_context(tc.tile_pool(name="psum", bufs=6, space="PSUM"))
        psum_s = ctx.enter_context(tc.tile_pool(name="psum_s", bufs=1, space="PSUM"))

        ident = const.tile([P, P], F32)
        make_identity(nc, ident)

        # ---- loads ------------------------------------------------------
        gw = const.tile([P, KT, E], F32R)
        for k in range(KT):
            nc.sync.dma_start(gw[:, k, :], _r(gate_w[k * P : (k + 1) * P, :]))
        gb8 = const.tile([E, 1], F32)
        nc.sync.dma_start(gb8, gate_b[:, None])
        eb = const.tile([E, H], F32R)
        nc.sync.dma_start(eb, _r(expert_b))

        # xT: h on partitions, t on free — straight (contiguous) DMA from the
        # host-transposed shard. GpSimd SWDGE queues, half-column chunks in
        # half-major order so the first gate matmul's operands land first.
        xT = [const.tile([P, TL], F32R, name=f"xT{k}") for k in range(KT)]
        xcw = TL // XC
        for c in range(XC):
            for k in range(KT):
                csl = slice(c * xcw, (c + 1) * xcw)
                nc.gpsimd.dma_start(
                    xT[k][:, csl], _r(x_shT[k * P : (k + 1) * P, csl])
                )

        # ---- gate -------------------------------------------------------
        ewT_raw = const.tile([E, TL], F32)    # exp(logits).T (unnormalized)
        ews = const.tile([P, MT, E], F32)     # per-token gate weight / S
        ewsT = const.tile([E, TL], F32R)      # ews transposed, for bias mm

        for h2 in range(2):
            hsl = slice(h2 * DH, (h2 + 1) * DH)
            pgT = psum_s.tile([E, DH], F32, tag="sm", bufs=2)
            for k in range(KT):
                nc.tensor.matmul(
                    pgT,
                    lhsT=gw[:, k, :],
                    rhs=xT[k][:, hsl],
                    start=(k == 0),
                    stop=(k == KT - 1),
                )
            # ewT = exp(logitsT + gate_b); gate_b is per-partition here
            nc.scalar.activation(
                ewT_raw[:, hsl], pgT, mybir.ActivationFunctionType.Exp, bias=gb8
            )

        for m in range(MT):
            msl = slice(m * P, (m + 1) * P)
            # ew[t, e] for this token tile via PE transpose
            ptw = psum_s.tile([P, E], F32, tag="sm", bufs=2)
            nc.tensor.transpose(ptw, ewT_raw[:, msl], ident[:E, :E])
            ssum = tmp.tile([P, 1], F32, tag="ssum")
            nc.vector.reduce_sum(ssum, ptw, axis=mybir.AxisListType.X)
            inv = tmp.tile([P, 1], F32, tag="inv")
            nc.vector.reciprocal(inv, ssum)
            nc.vector.tensor_scalar_mul(ews[:, m, :], ptw, inv)
            # back-transpose the normalized weights for the bias matmul
            ptb = psum_s.tile([E, P], F32, tag="sm", bufs=2)
            nc.tensor.transpose(ptb, ews[:, m, :], ident)
            nc.vector.tensor_copy(ewsT[:, msl], _r(ptb))

        # ---- bias seed: acc = ews @ expert_b ---------------------------
        acc = [accp.tile([P, H], F32, name=f"acc{m}") for m in range(MT)]
        for m in range(MT):
            msl = slice(m * P, (m + 1) * P)
            for n in range(ND):
                nsl = slice(n * DH, (n + 1) * DH)
                pb = psum.tile([P, DH], F32, tag="ps")
                nc.tensor.matmul(
                    pb, lhsT=ewsT[:, msl], rhs=eb[:, nsl], start=True, stop=True
                )
                nc.vector.tensor_copy(acc[m][:, nsl], pb)

        # ---- experts ----------------------------------------------------
        ocw = DH // OC
        for e in range(E):
            wsb = wpool.tile([P, KT, H], F32R, tag="w")
            wcw = H // WC
            for c in range(WC):
                for k in range(KT):
                    csl = slice(c * wcw, (c + 1) * wcw)
                    nc.sync.dma_start(
                        wsb[:, k, csl],
                        _r(expert_w[e, k * P : (k + 1) * P, csl]),
                    )
            last = e == E - 1
            for n in range(ND):
                nsl = slice(n * DH, (n + 1) * DH)
                for m in range(MT):
                    msl = slice(m * P, (m + 1) * P)
                    ps = psum.tile([P, DH], F32, tag="ps")
                    for k in range(KT):
                        nc.tensor.matmul(
                            ps,
                            lhsT=xT[k][:, msl],
                            rhs=wsb[:, k, nsl],
                            start=(k == 0),
                            stop=(k == KT - 1),
                        )
                    # evict scaled by normalized gate weight; alternate the
                    # scale between ACT and DVE so neither engine saturates
                    t = tmp.tile([P, DH], F32, tag="evict")
                    if (m + n) % 2 == 0:
                        nc.scalar.mul(t, ps, ews[:, m, e : e + 1])
                    else:
                        nc.vector.tensor_scalar_mul(t, ps, ews[:, m, e : e + 1])
                    nc.vector.tensor_add(acc[m][:, nsl], acc[m][:, nsl], t)
                    if last:
                        for c in range(OC):
                            osl = slice(
                                n * DH + c * ocw, n * DH + (c + 1) * ocw
                            )
                            nc.sync.dma_start(
                                out_sh[m * P : (m + 1) * P, osl],
                                acc[m][:, osl],
                            )

    nc.compile()
    return nc


def kernel(**inputs) -> np.ndarray:
    global LAST_RESULT
    x = np.asarray(inputs["x"], dtype=np.float32).reshape(T, H)
    gw = np.ascontiguousarray(np.asarray(inputs["gate_w"], dtype=np.float32))
    gb = np.ascontiguousarray(np.asarray(inputs["gate_b"], dtype=np.float32))
    ew = np.ascontiguousarray(np.asarray(inputs["expert_w"], dtype=np.float32))
    eb = np.ascontiguousarray(np.asarray(inputs["expert_b"], dtype=np.float32))

    if "nc" not in _CACHE:
        _CACHE["nc"] = _build_moe_nc()
    nc = _CACHE["nc"]

    in_maps = [
        {
            "x_shT": np.ascontiguousarray(x[c * TL : (c + 1) * TL].T),
            "gate_w": gw,
            "gate_b": gb,
            "expert_w": ew,
            "expert_b": eb,
        }
        for c in range(N_CORES)
    ]
    res = run_bass_kernel_spmd(
        nc,
        in_maps,
        core_ids=list(range(N_CORES)),
        trace=bool(int(os.environ.get("MOE_TRACE", "0"))),
    )
    LAST_RESULT = res
    out = np.concatenate([res.results[c]["out_sh"] for c in range(N_CORES)], axis=0)
    return out.reshape(B, S, H)


# revision 10
# speedup vs baseline: 1.0808x; 1.0054x over previous
"""Dense MoE (BasicMoE) Trainium2 Bass kernel.

Problem (hardcoded): x [4, 2048, 1024] f32, gate_w [1024, 8], gate_b [8],
expert_w [8, 1024, 1024], expert_b [8, 1024].

    tok = x.reshape(T, H)
    w   = softmax(tok @ gate_w + gate_b)           # [T, E]
    eo  = einsum('th,ehd->ted', tok, expert_w) + expert_b
    out = einsum('te,ted->td', w, eo)              # [T, H]

Sharding: tokens split across 8 cores (data parallel), weights replicated.

Per-core algorithm (T_l = 1024 tokens). The TensorEngine contracts along
the partition dim, so the activations are needed h-major (xT); that
transpose is pure data layout, done host-side when sharding.

  1. Gate, in transposed [e, t] layout: logitsT = gate_w.T @ x.T via
     matmuls with gate_w slices stationary (8-wide LDWEIGHTS);
     ewT = exp(logitsT + gate_b) with gate_b as a natural per-partition
     ACT bias. Small PE transposes give ew in [t, e] layout, where
     S = sum_e ew and ews = ew * (1/S) are per-partition ops. softmax's
     division is linear in the combine, so it is folded into the gate
     weights and nothing needs normalizing at the end. ews is transposed
     back (ewsT) for the bias term.
  2. acc[t,d] seeded with the bias term sum_e ews[t,e]*b_e[d] (K=8 matmul
     of ewsT against expert_b).
  3. For each expert: y_e = xT.T @ W_e accumulated over k in PSUM
     (float32r matmuls: full PE rate), evicted by the scalar engine as
     y_e * ews[:,e] (per-partition scale) and added into acc by DVE.
  4. acc IS the output: DMA out per 512-wide half as soon as the last
     expert's contribution lands.
"""

import os
from contextlib import ExitStack

import numpy as np

import concourse.bass as bass
import concourse.tile as tile
from concourse import bacc, mybir
from concourse.bass_utils import run_bass_kernel_spmd
from concourse.masks import make_identity

B, S, H, E = 4, 2048, 1024, 8
T = B * S
N_CORES = 8
TL = T // N_CORES          # tokens per core = 1024
P = 128                    # SBUF partitions
KT = H // P                # 8 contraction tiles
MT = TL // P               # 8 token tiles per core
DH = 512                   # matmul moving free-dim (fp32 PSUM bank)
ND = H // DH               # 2 d-halves
XC = 2                     # x DMA column chunks (queue parallelism)
WC = 2                     # expert_w DMA column chunks
OC = 2                     # output DMA column chunks per (m, half)

F32 = mybir.dt.float32
F32R = mybir.dt.float32r

_CACHE = {}
LAST_RESULT = None


def _r(ap):
    """Bitcast an f32 AP to float32r (same bits; PE rounds internally)."""
    return ap.bitcast(F32R)


def _build_moe_nc():
    nc = bacc.Bacc(
        "TRN2",
        target_bir_lowering=False,
        debug=False,
        enable_asserts=False,
        num_devices=N_CORES,
    )

    x_shT = nc.dram_tensor("x_shT", [H, TL], F32, kind="ExternalInput").ap()
    gate_w = nc.dram_tensor("gate_w", [H, E], F32, kind="ExternalInput").ap()
    gate_b = nc.dram_tensor("gate_b", [E], F32, kind="ExternalInput").ap()
    expert_w = nc.dram_tensor("expert_w", [E, H, H], F32, kind="ExternalInput").ap()
    expert_b = nc.dram_tensor("expert_b", [E, H], F32, kind="ExternalInput").ap()
    out_sh = nc.dram_tensor("out_sh", [TL, H], F32, kind="ExternalOutput").ap()

    with tile.TileContext(nc) as tc, ExitStack() as ctx:
        const = ctx.enter_context(tc.tile_pool(name="const", bufs=1))
        wpool = ctx.enter_context(tc.tile_pool(name="wpool", bufs=2))
        accp = ctx.enter_context(tc.tile_pool(name="accp", bufs=1))
        tmp = ctx.enter_context(tc.tile_pool(name="tmp", bufs=6))
        psum = ctx.enter_context(tc.tile_pool(name="psum", bufs=6, space="PSUM"))
        psum_s = ctx.enter_context(tc.tile_pool(name="psum_s", bufs=1, space="PSUM"))

        ident = const.tile([P, P], F32)
        make_identity(nc, ident)

        # ---- loads ------------------------------------------------------
        gw = const.tile([P, KT, E], F32R)
        for k in range(KT):
            nc.sync.dma_start(gw[:, k, :], _r(gate_w[k * P : (k + 1) * P, :]))
        gb8 = const.tile([E, 1], F32)
        nc.sync.dma_start(gb8, gate_b[:, None])
        eb = const.tile([E, H], F32R)
        nc.sync.dma_start(eb, _r(expert_b))

        # xT: h on partitions, t on free — straight (contiguous) DMA from the
        # host-transposed shard. GpSimd SWDGE queues, half-column chunks in
        # half-major order so the first gate matmul's operands land first.
        xT = [const.tile([P, TL], F32R, name=f"xT{k}") for k in range(KT)]
        xcw = TL // XC
        for c in range(XC):
            for k in range(KT):
                csl = slice(c * xcw, (c + 1) * xcw)
                nc.gpsimd.dma_start(
                    xT[k][:, csl], _r(x_shT[k * P : (k + 1) * P, csl])
                )

        # ---- gate -------------------------------------------------------
        ewT_raw = const.tile([E, TL], F32)    # exp(logits).T (unnormalized)
        ews = const.tile([P, MT, E], F32)     # per-token gate weight / S
        ewsT = const.tile([E, TL], F32R)      # ews transposed, for bias mm

        for h2 in range(2):
            hsl = slice(h2 * DH, (h2 + 1) * DH)
            pgT = psum_s.tile([E, DH], F32, tag="sm", bufs=2)
            for k in range(KT):
                nc.tensor.matmul(
                    pgT,
                    lhsT=gw[:, k, :],
                    rhs=xT[k][:, hsl],
                    start=(k == 0),
                    stop=(k == KT - 1),
                )
            # ewT = exp(logitsT + gate_b); gate_b is per-partition here
            nc.scalar.activation(
                ewT_raw[:, hsl], pgT, mybir.ActivationFunctionType.Exp, bias=gb8
            )

        for m in range(MT):
            msl = slice(m * P, (m + 1) * P)
            # ew[t, e] for this token tile via PE transpose
            ptw = psum_s.tile([P, E], F32, tag="sm", bufs=2)
            nc.tensor.transpose(ptw, ewT_raw[:, msl], ident[:E, :E])
            ssum = tmp.tile([P, 1], F32, tag="ssum")
            nc.vector.reduce_sum(ssum, ptw, axis=mybir.AxisListType.X)
            inv = tmp.tile([P, 1], F32, tag="inv")
            nc.vector.reciprocal(inv, ssum)
            nc.vector.tensor_scalar_mul(ews[:, m, :], ptw, inv)
            # back-transpose the normalized weights for the bias matmul
            ptb = psum_s.tile([E, P], F32, tag="sm", bufs=2)
            nc.tensor.transpose(ptb, ews[:, m, :], ident)
            nc.vector.tensor_copy(ewsT[:, msl], _r(ptb))

        # ---- bias seed: acc = ews @ expert_b ---------------------------
        acc = [accp.tile([P, H], F32, name=f"acc{m}") for m in range(MT)]
        for m in range(MT):
            msl = slice(m * P, (m + 1) * P)
            for n in range(ND):
                nsl = slice(n * DH, (n + 1) * DH)
                pb = psum.tile([P, DH], F32, tag="ps")
                nc.tensor.matmul(
                    pb, lhsT=ewsT[:, msl], rhs=eb[:, nsl], start=True, stop=True
                )
                nc.vector.tensor_copy(acc[m][:, nsl], pb)

        # ---- experts ----------------------------------------------------
        ocw = DH // OC
        for e in range(E):
            wsb = wpool.tile([P, KT, H], F32R, tag="w")
            # e=0 is latency-critical (PE is waiting): split across both
            # HWDGE and SWDGE queue sets in small chunks. Steady state uses
            # the sync queues only.
            ewc = 4 if e == 0 else WC
            wcw = H // ewc
            for c in range(ewc):
                for k in range(KT):
                    csl = slice(c * wcw, (c + 1) * wcw)
                    eng = nc.gpsimd if (e == 0 and k % 2 == 1) else nc.sync
                    eng.dma_start(
                        wsb[:, k, csl],
                        _r(expert_w[e, k * P : (k + 1) * P, csl]),
                    )
            last = e == E - 1
            for n in range(ND):
                nsl = slice(n * DH, (n + 1) * DH)
                for m in range(MT):
                    msl = slice(m * P, (m + 1) * P)
                    ps = psum.tile([P, DH], F32, tag="ps")
                    for k in range(KT):
                        nc.tensor.matmul(
                            ps,
                            lhsT=xT[k][:, msl],
                            rhs=wsb[:, k, nsl],
                            start=(k == 0),
                            stop=(k == KT - 1),
                        )
                    # evict scaled by normalized gate weight; alternate the
                    # scale between ACT and DVE so neither engine saturates
                    t = tmp.tile([P, DH], F32, tag="evict")
                    if (m + n) % 2 == 0:
                        nc.scalar.mul(t, ps, ews[:, m, e : e + 1])
                    else:
                        nc.vector.tensor_scalar_mul(t, ps, ews[:, m, e : e + 1])
                    nc.vector.tensor_add(acc[m][:, nsl], acc[m][:, nsl], t)
                    if last:
                        for c in range(OC):
                            osl = slice(
                                n * DH + c * ocw, n * DH + (c + 1) * ocw
                            )
                            nc.sync.dma_start(
                                out_sh[m * P : (m + 1) * P, osl],
                                acc[m][:, osl],
                            )

    nc.compile()
    return nc


def kernel(**inputs) -> np.ndarray:
    global LAST_RESULT
    x = np.asarray(inputs["x"], dtype=np.float32).reshape(T, H)
    gw = np.ascontiguousarray(np.asarray(inputs["gate_w"], dtype=np.float32))
    gb = np.ascontiguousarray(np.asarray(inputs["gate_b"], dtype=np.float32))
    ew = np.ascontiguousarray(np.asarray(inputs["expert_w"], dtype=np.float32))
    eb = np.ascontiguousarray(np.asarray(inputs["expert_b"], dtype=np.float32))

    if "nc" not in _CACHE:
        _CACHE["nc"] = _build_moe_nc()
    nc = _CACHE["nc"]

    in_maps = [
        {
            "x_shT": np.ascontiguousarray(x[c * TL : (c + 1) * TL].T),
            "gate_w": gw,
            "gate_b": gb,
            "expert_w": ew,
            "expert_b": eb,
        }
        for c in range(N_CORES)
    ]
    res = run_bass_kernel_spmd(
        nc,
        in_maps,
        core_ids=list(range(N_CORES)),
        trace=bool(int(os.environ.get("MOE_TRACE", "0"))),
    )
    LAST_RESULT = res
    out = np.concatenate([res.results[c]["out_sh"] for c in range(N_CORES)], axis=0)
    return out.reshape(B, S, H)


# revision 11
# speedup vs baseline: 1.1396x; 1.0544x over previous
"""Dense MoE (BasicMoE) Trainium2 Bass kernel.

Problem (hardcoded): x [4, 2048, 1024] f32, gate_w [1024, 8], gate_b [8],
expert_w [8, 1024, 1024], expert_b [8, 1024].

    tok = x.reshape(T, H)
    w   = softmax(tok @ gate_w + gate_b)           # [T, E]
    eo  = einsum('th,ehd->ted', tok, expert_w) + expert_b
    out = einsum('te,ted->td', w, eo)              # [T, H]

Sharding: tokens split across 8 cores (data parallel), weights replicated.

Per-core algorithm (T_l = 1024 tokens). The TensorEngine contracts along
the partition dim, so the activations are needed h-major (xT); that
transpose is pure data layout, done host-side when sharding.

  1. Gate, in transposed [e, t] layout: logitsT = gate_w.T @ x.T via
     matmuls with gate_w slices stationary (8-wide LDWEIGHTS);
     ewT = exp(logitsT + gate_b) with gate_b as a natural per-partition
     ACT bias. Small PE transposes give ew in [t, e] layout, where
     S = sum_e ew and ews = ew * (1/S) are per-partition ops. softmax's
     division is linear in the combine, so it is folded into the gate
     weights and nothing needs normalizing at the end. ews is transposed
     back (ewsT) for the bias term.
  2. acc[t,d] seeded with the bias term sum_e ews[t,e]*b_e[d] (K=8 matmul
     of ewsT against expert_b).
  3. For each expert: y_e = xT.T @ W_e accumulated over k in PSUM
     (float32r matmuls: full PE rate), evicted by the scalar engine as
     y_e * ews[:,e] (per-partition scale) and added into acc by DVE.
  4. acc IS the output: DMA out per 512-wide half as soon as the last
     expert's contribution lands.
"""

import os
from contextlib import ExitStack

import numpy as np

import concourse.bass as bass
import concourse.tile as tile
from concourse import bacc, mybir
from concourse.bass_utils import run_bass_kernel_spmd
from concourse.masks import make_identity

B, S, H, E = 4, 2048, 1024, 8
T = B * S
N_CORES = 8
TL = T // N_CORES          # tokens per core = 1024
P = 128                    # SBUF partitions
KT = H // P                # 8 contraction tiles
MT = TL // P               # 8 token tiles per core
DH = 512                   # matmul moving free-dim (fp32 PSUM bank)
ND = H // DH               # 2 d-halves
XC = 2                     # x DMA column chunks (queue parallelism)
WC = 2                     # expert_w DMA column chunks
OC = 2                     # output DMA column chunks per (m, half)

F32 = mybir.dt.float32
F32R = mybir.dt.float32r
BF16 = mybir.dt.bfloat16

_CACHE = {}
LAST_RESULT = None


def _r(ap):
    """Bitcast an f32 AP to float32r (same bits; PE rounds internally)."""
    return ap.bitcast(F32R)


def _build_moe_nc():
    nc = bacc.Bacc(
        "TRN2",
        target_bir_lowering=False,
        debug=False,
        enable_asserts=False,
        num_devices=N_CORES,
    )

    x_shT = nc.dram_tensor("x_shT", [H, TL], BF16, kind="ExternalInput").ap()
    gate_w = nc.dram_tensor("gate_w", [H, E], BF16, kind="ExternalInput").ap()
    gate_b = nc.dram_tensor("gate_b", [E], F32, kind="ExternalInput").ap()
    expert_w = nc.dram_tensor("expert_w", [E, H, H], BF16, kind="ExternalInput").ap()
    expert_b = nc.dram_tensor("expert_b", [E, H], F32, kind="ExternalInput").ap()
    out_sh = nc.dram_tensor("out_sh", [TL, H], F32, kind="ExternalOutput").ap()

    with tile.TileContext(nc) as tc, ExitStack() as ctx:
        const = ctx.enter_context(tc.tile_pool(name="const", bufs=1))
        wpool = ctx.enter_context(tc.tile_pool(name="wpool", bufs=2))
        accp = ctx.enter_context(tc.tile_pool(name="accp", bufs=1))
        tmp = ctx.enter_context(tc.tile_pool(name="tmp", bufs=6))
        psum = ctx.enter_context(tc.tile_pool(name="psum", bufs=6, space="PSUM"))
        psum_s = ctx.enter_context(tc.tile_pool(name="psum_s", bufs=1, space="PSUM"))

        ident = const.tile([P, P], F32)
        make_identity(nc, ident)

        # ---- loads ------------------------------------------------------
        gw = const.tile([P, KT, E], BF16)
        for k in range(KT):
            nc.sync.dma_start(gw[:, k, :], gate_w[k * P : (k + 1) * P, :])
        gb8 = const.tile([E, 1], F32)
        nc.sync.dma_start(gb8, gate_b[:, None])
        eb = const.tile([E, H], F32R)
        nc.sync.dma_start(eb, _r(expert_b))

        # xT: h on partitions, t on free — straight (contiguous) DMA from the
        # host-transposed shard. GpSimd SWDGE queues, half-column chunks in
        # half-major order so the first gate matmul's operands land first.
        xT = [const.tile([P, TL], BF16, name=f"xT{k}") for k in range(KT)]
        xcw = TL // XC
        for c in range(XC):
            for k in range(KT):
                csl = slice(c * xcw, (c + 1) * xcw)
                nc.gpsimd.dma_start(
                    xT[k][:, csl], x_shT[k * P : (k + 1) * P, csl]
                )

        # ---- gate -------------------------------------------------------
        ewT_raw = const.tile([E, TL], F32)    # exp(logits).T (unnormalized)
        ews = const.tile([P, MT, E], F32)     # per-token gate weight / S
        ewsT = const.tile([E, TL], F32R)      # ews transposed, for bias mm

        for h2 in range(2):
            hsl = slice(h2 * DH, (h2 + 1) * DH)
            pgT = psum_s.tile([E, DH], F32, tag="sm", bufs=2)
            for k in range(KT):
                nc.tensor.matmul(
                    pgT,
                    lhsT=gw[:, k, :],
                    rhs=xT[k][:, hsl],
                    start=(k == 0),
                    stop=(k == KT - 1),
                )
            # ewT = exp(logitsT + gate_b); gate_b is per-partition here
            nc.scalar.activation(
                ewT_raw[:, hsl], pgT, mybir.ActivationFunctionType.Exp, bias=gb8
            )

        for m in range(MT):
            msl = slice(m * P, (m + 1) * P)
            # ew[t, e] for this token tile via PE transpose
            ptw = psum_s.tile([P, E], F32, tag="sm", bufs=2)
            nc.tensor.transpose(ptw, ewT_raw[:, msl], ident[:E, :E])
            ssum = tmp.tile([P, 1], F32, tag="ssum")
            nc.vector.reduce_sum(ssum, ptw, axis=mybir.AxisListType.X)
            inv = tmp.tile([P, 1], F32, tag="inv")
            nc.vector.reciprocal(inv, ssum)
            nc.vector.tensor_scalar_mul(ews[:, m, :], ptw, inv)
            # back-transpose the normalized weights for the bias matmul
            ptb = psum_s.tile([E, P], F32, tag="sm", bufs=2)
            nc.tensor.transpose(ptb, ews[:, m, :], ident)
            nc.vector.tensor_copy(ewsT[:, msl], _r(ptb))

        # ---- bias seed: acc = ews @ expert_b ---------------------------
        acc = [accp.tile([P, H], F32, name=f"acc{m}") for m in range(MT)]
        for m in range(MT):
            msl = slice(m * P, (m + 1) * P)
            for n in range(ND):
                nsl = slice(n * DH, (n + 1) * DH)
                pb = psum.tile([P, DH], F32, tag="ps")
                nc.tensor.matmul(
                    pb, lhsT=ewsT[:, msl], rhs=eb[:, nsl], start=True, stop=True
                )
                nc.vector.tensor_copy(acc[m][:, nsl], pb)

        # ---- experts ----------------------------------------------------
        ocw = DH // OC
        for e in range(E):
            wsb = wpool.tile([P, KT, H], BF16, tag="w")
            # e=0 is latency-critical (PE is waiting): split across both
            # HWDGE and SWDGE queue sets in small chunks. Steady state uses
            # the sync queues only.
            ewc = 4 if e == 0 else WC
            wcw = H // ewc
            for c in range(ewc):
                for k in range(KT):
                    csl = slice(c * wcw, (c + 1) * wcw)
                    eng = nc.gpsimd if (e == 0 and k % 2 == 1) else nc.sync
                    eng.dma_start(
                        wsb[:, k, csl],
                        expert_w[e, k * P : (k + 1) * P, csl],
                    )
            last = e == E - 1
            for n in range(ND):
                nsl = slice(n * DH, (n + 1) * DH)
                for m in range(MT):
                    msl = slice(m * P, (m + 1) * P)
                    ps = psum.tile([P, DH], F32, tag="ps")
                    for k in range(KT):
                        nc.tensor.matmul(
                            ps,
                            lhsT=xT[k][:, msl],
                            rhs=wsb[:, k, nsl],
                            start=(k == 0),
                            stop=(k == KT - 1),
                        )
                    # evict scaled by normalized gate weight; alternate the
                    # scale between ACT and DVE so neither engine saturates
                    t = tmp.tile([P, DH], F32, tag="evict")
                    if (m + n) % 2 == 0:
                        nc.scalar.mul(t, ps, ews[:, m, e : e + 1])
                    else:
                        nc.vector.tensor_scalar_mul(t, ps, ews[:, m, e : e + 1])
                    nc.vector.tensor_add(acc[m][:, nsl], acc[m][:, nsl], t)
                    if last:
                        noc = OC * 2 if m == MT - 1 else OC
                        for c in range(noc):
                            ocw2 = DH // noc
                            osl = slice(
                                n * DH + c * ocw2, n * DH + (c + 1) * ocw2
                            )
                            nc.sync.dma_start(
                                out_sh[m * P : (m + 1) * P, osl],
                                acc[m][:, osl],
                            )

    nc.compile()
    return nc


def kernel(**inputs) -> np.ndarray:
    global LAST_RESULT
    import ml_dtypes

    bf16 = ml_dtypes.bfloat16
    x = np.asarray(inputs["x"], dtype=np.float32).reshape(T, H)
    gw = np.ascontiguousarray(np.asarray(inputs["gate_w"], dtype=np.float32).astype(bf16))
    gb = np.ascontiguousarray(np.asarray(inputs["gate_b"], dtype=np.float32))
    ew = np.ascontiguousarray(np.asarray(inputs["expert_w"], dtype=np.float32).astype(bf16))
    eb = np.ascontiguousarray(np.asarray(inputs["expert_b"], dtype=np.float32))

    if "nc" not in _CACHE:
        _CACHE["nc"] = _build_moe_nc()
    nc = _CACHE["nc"]

    in_maps = [
        {
            "x_shT": np.ascontiguousarray(x[c * TL : (c + 1) * TL].T.astype(bf16)),
            "gate_w": gw,
            "gate_b": gb,
            "expert_w": ew,
            "expert_b": eb,
        }
        for c in range(N_CORES)
    ]
    res = run_bass_kernel_spmd(
        nc,
        in_maps,
        core_ids=list(range(N_CORES)),
        trace=bool(int(os.environ.get("MOE_TRACE", "0"))),
    )
    LAST_RESULT = res
    out = np.concatenate([res.results[c]["out_sh"] for c in range(N_CORES)], axis=0)
    return out.reshape(B, S, H)
